# revision 17
# baseline (speedup 1.0000x reference)
"""Trainium2 Bass kernel for nn_CGAPB_84052509983238 (EGNN coarse-graining
autoencoder: pool EGNN -> softmax projection + losses -> bead coords ->
depool EGNN -> mean_x).

Sharding: data-parallel over the batch. 128 graphs / 8 cores = 16 graphs per
core; each core is fully independent (losses are partial-summed on device,
combined on host; mean_x is concatenated).

Edge layout on device: per graph the 4096 fully-connected edges are ordered
k = r*64 + s (receiver-major). The edge-MLP input M1 is assembled entirely in
PSUM by a single K=100 matmul against a mostly-static indicator rhs:
  rhs rows  0..63 : ind_s[j,k]   = [k%64 == j]           (static)
  rhs rows 64..95 : ind_r32[j,k] = [(k%2048)//64 == j]   (static)
  rhs rows 96..98 : onehot(edge_attr)                     (per graph)
  rhs row  99     : Grow[k] = (X @ X.T)[r,s]              (per graph-layer)
matched against lhsT rows [B'(64); A'half(32); EW(3); -2*wd(1)] where
A' = h@We1_r + nx2*wd, B' = h@We1_s + nx2*wd. This reproduces
M1 = h_r@We1_r + h_s@We1_s + d2*wd + emb_e@We1_e with d2 expanded as
nx2[s] + nx2[r] - 2*(x_s . x_r).
"""
import os
import sys

sys.path.insert(0, "/opt/trn_rl_repo")

import numpy as np

B, N, NB, NR, H, NL = 128, 64, 16, 32, 128, 2
NCORES = 8
G = B // NCORES  # graphs per core
E = N * N  # pool edges per graph
EB = NB * NB  # depool edges per graph
CH = 512  # band chunk (psum bank)
NCH = E // CH  # 8 chunks per pool graph-layer

_PROGRAM = None  # cached compiled program -- compile once per process
_LAST_RESULT = None  # BassKernelResults of the most recent run (for test.py)


# --------------------------------------------------------------------------
# host-side packing
# --------------------------------------------------------------------------

def _np32(a):
    return np.ascontiguousarray(np.asarray(a, dtype=np.float32))


def _col(v):
    return _np32(v).reshape(-1, 1)


def _pack_egnn(params, prefix, fold_edge0_into_be1):
    """Flatten one EGNN's params into the DRAM-tensor dict."""
    out = {}
    edge_emb = _np32(params["edge_emb"])
    for l, lp in enumerate(params["layers"]):
        We1 = _np32(lp["We1"])  # (385, H) rows: [h_r(128); h_s(128); d2(1); e(128)]
        we1r, we1s = We1[:H], We1[H : 2 * H]
        wd = We1[2 * H]  # (128,)
        we1e = We1[2 * H + 1 :]
        EW = edge_emb @ we1e  # (n_edge_types, 128)
        be1 = _np32(lp["be1"])
        if fold_edge0_into_be1:
            be1 = be1 + EW[0]
        else:
            ewwd = np.concatenate([EW, (-2.0 * wd)[None, :]], axis=0)  # (4,128)
            out[f"{prefix}{l}_ewwd"] = _np32(ewwd)
        out[f"{prefix}{l}_ndwd"] = _np32((-2.0 * wd)[None, :])  # (1,128)
        out[f"{prefix}{l}_we1r"] = we1r
        out[f"{prefix}{l}_we1s"] = we1s
        out[f"{prefix}{l}_wd3"] = _np32(np.broadcast_to(wd, (3, H)))
        out[f"{prefix}{l}_we2"] = _np32(lp["We2"])
        out[f"{prefix}{l}_wc"] = _np32(lp["Wc"])  # (128,1)
        out[f"{prefix}{l}_be1"] = _col(be1)
        out[f"{prefix}{l}_be2"] = _col(lp["be2"])
        out[f"{prefix}{l}_bn1"] = _col(lp["bn1"])
        out[f"{prefix}{l}_bn2"] = _col(lp["bn2"])
        out[f"{prefix}{l}_wn1a"] = _np32(lp["Wn1"][:H])
        out[f"{prefix}{l}_wn1b"] = _np32(lp["Wn1"][H:])
        out[f"{prefix}{l}_wn2"] = _np32(lp["Wn2"])
    return out


def _static_masks():
    k = np.arange(E)
    indp = np.zeros((96, E), np.float32)
    for j in range(64):
        indp[j] = (k % 64) == j
    for j in range(32):
        indp[64 + j] = ((k % 2048) // 64) == j
    kb = np.arange(EB)
    indd = np.zeros((48, EB), np.float32)
    for j in range(16):
        indd[j] = (kb // 16) == j
    for j in range(16):
        indd[32 + j] = (kb % 16) == j
    return indp, indd


def _host_pack(inputs, g_count=G, ncores=NCORES):
    """Build the per-core input maps."""
    x = _np32(inputs["x"]).reshape(B, N, 3)
    node_feats = np.asarray(inputs["node_feats"]).astype(np.float32).reshape(B, N)
    edge_attr = np.asarray(inputs["edge_attr"]).astype(np.float32).reshape(B, N, N)
    # reference edge k_ref = s*N + r  ->  grid[s, r]; we want attrg[r, s]
    attrg = np.ascontiguousarray(edge_attr.transpose(0, 2, 1)).reshape(B, E)
    dist = _np32(inputs["distances"])

    shared = {}
    shared.update(_pack_egnn(inputs["pool_params"], "p", fold_edge0_into_be1=False))
    shared.update(_pack_egnn(inputs["depool_params"], "d", fold_edge0_into_be1=True))
    shared["p_nemb"] = _np32(inputs["pool_params"]["node_emb"])  # (4,128)
    shared["p_wout"] = _np32(inputs["pool_params"]["Wout"])  # (128,16)
    shared["d_nembT"] = _np32(np.asarray(inputs["depool_params"]["node_emb"]).T)
    shared["d_wout"] = _np32(inputs["depool_params"]["Wout"])  # (128,32)
    shared["d_bout"] = _col(inputs["depool_params"]["bout"])  # (32,1)
    indp, indd = _static_masks()
    shared["indp"] = indp
    shared["indd"] = indd
    shared["ident64"] = np.eye(64, dtype=np.float32)
    shared["ones64"] = np.ones((64, 1), np.float32)

    x4 = np.ones((B, N, 4), np.float32)
    x4[:, :, :3] = x
    xt4 = np.ones((B, 4, N), np.float32)
    xt4[:, :3, :] = x.transpose(0, 2, 1)

    in_maps = []
    for c in range(ncores):
        sl = slice(c * g_count, (c + 1) * g_count)
        m = dict(shared)
        m["x4"] = np.ascontiguousarray(x4[sl])
        m["xt4"] = np.ascontiguousarray(xt4[sl])
        m["nf"] = np.ascontiguousarray(node_feats[sl].reshape(1, g_count * N))
        m["attrg"] = np.ascontiguousarray(attrg[sl])
        m["dist"] = np.ascontiguousarray(dist[sl])
        in_maps.append(m)
    return in_maps


# --------------------------------------------------------------------------
# device program
# --------------------------------------------------------------------------

def build_program(g_count=G):
    import concourse.bacc as bacc
    import concourse.tile as tile
    from concourse import mybir

    F32 = mybir.dt.float32
    I32 = mybir.dt.int32
    AF = mybir.ActivationFunctionType
    ALU = mybir.AluOpType
    AX = mybir.AxisListType

    nc = bacc.Bacc("TRN2", target_bir_lowering=False, debug=False,
                   num_devices=NCORES)

    def din(name, shape, dt=F32):
        return nc.dram_tensor(name, list(shape), dt, kind="ExternalInput").ap()

    def dout(name, shape, dt=F32):
        return nc.dram_tensor(name, list(shape), dt, kind="ExternalOutput").ap()

    dr = {}
    dr["x4"] = din("x4", (g_count, N, 4))
    dr["xt4"] = din("xt4", (g_count, 4, N))
    dr["nf"] = din("nf", (1, g_count * N))
    dr["attrg"] = din("attrg", (g_count, E))
    dr["dist"] = din("dist", (g_count, N, N))
    dr["indp"] = din("indp", (96, E))
    dr["indd"] = din("indd", (48, EB))
    dr["ident64"] = din("ident64", (64, 64))
    dr["ones64"] = din("ones64", (64, 1))
    dr["p_nemb"] = din("p_nemb", (4, H))
    dr["p_wout"] = din("p_wout", (H, NB))
    dr["d_nembT"] = din("d_nembT", (H, NB))
    dr["d_wout"] = din("d_wout", (H, NR))
    dr["d_bout"] = din("d_bout", (NR, 1))
    for pre in ("p", "d"):
        for l in range(NL):
            for nm, shp in [("we1r", (H, H)), ("we1s", (H, H)), ("wd3", (3, H)),
                            ("we2", (H, H)), ("wc", (H, 1)), ("be1", (H, 1)),
                            ("be2", (H, 1)), ("bn1", (H, 1)), ("bn2", (H, 1)),
                            ("wn1a", (H, H)), ("wn1b", (H, H)), ("wn2", (H, H)),
                            ("ndwd", (1, H))]:
                dr[f"{pre}{l}_{nm}"] = din(f"{pre}{l}_{nm}", shp)
            if pre == "p":
                dr[f"p{l}_ewwd"] = din(f"p{l}_ewwd", (4, H))
    out_mx = dout("mean_x", (g_count, NR, 3))
    out_sc = dout("scalars", (1, 3))

    from contextlib import ExitStack

    with tile.TileContext(nc) as tc, ExitStack() as ctx:
        cpool = ctx.enter_context(tc.tile_pool(name="consts", bufs=1))
        pg = ctx.enter_context(tc.tile_pool(name="graph", bufs=3))
        wide = ctx.enter_context(tc.tile_pool(name="wide", bufs=2))
        band = ctx.enter_context(tc.tile_pool(name="band", bufs=4))
        ps_band = ctx.enter_context(tc.tile_pool(name="psb", bufs=4, space="PSUM"))
        ps_node = ctx.enter_context(tc.tile_pool(name="psn", bufs=2, space="PSUM"))
        ps_w = ctx.enter_context(tc.tile_pool(name="psw", bufs=2, space="PSUM"))

        # ---- load constants/params into SBUF --------------------------------
        sb = {}
        for name, ap in dr.items():
            if name in ("x4", "xt4", "nf", "attrg", "dist", "indp", "indd"):
                continue
            t = cpool.tile(list(ap.shape), ap.dtype, tag=name)
            nc.sync.dma_start(out=t[:], in_=ap[:])
            sb[name] = t

        # rhs slots for the pool band matmul (100 x 4096); rows 0..95 static
        rhs_slots = []
        for i in range(2):
            t = cpool.tile([100, E], F32, tag=f"rhs{i}")
            nc.sync.dma_start(out=t[0:96, :], in_=dr["indp"][:])
            rhs_slots.append(t)
        # depool rhs slots (48 x 256); rows 0..15, 32..47 static, 17..31 zero
        rhsd_slots = []
        for i in range(2):
            t = cpool.tile([48, EB], F32, tag=f"rhsd{i}")
            nc.sync.dma_start(out=t[:], in_=dr["indd"][:])
            rhsd_slots.append(t)

        # iota columns
        _ccols = {}

        def constcol(val):
            if val not in _ccols:
                t = cpool.tile([128, 1], F32, tag=f"cc{len(_ccols)}")
                nc.vector.memset(t[:], float(val))
                _ccols[val] = t
            return _ccols[val]

        iota4i = cpool.tile([4, 1], I32, tag="iota4i")
        nc.gpsimd.iota(iota4i[:], pattern=[[0, 1]], base=0, channel_multiplier=1)
        iota4 = cpool.tile([4, 1], F32, tag="iota4")
        nc.vector.tensor_copy(out=iota4[:], in_=iota4i[:])
        iota3 = iota4[0:3, :]

        # loss accumulator columns: [link(g_count) | diag | ent]
        scal = cpool.tile([64, 3 * g_count], F32, tag="scal")
        nc.vector.memset(scal[:], 0.0)

        # initial pool node features for all graphs: hT_all (128, G*64)
        nf_bc = cpool.tile([4, g_count * N], F32, tag="nfbc")
        nc.sync.dma_start(out=nf_bc[:], in_=dr["nf"].to_broadcast((4, g_count * N)))
        oh4 = cpool.tile([4, g_count * N], F32, tag="oh4")
        nc.vector.tensor_scalar(out=oh4[:], in0=nf_bc[:], scalar1=iota4[:],
                                scalar2=None, op0=ALU.is_equal)
        hT_all = cpool.tile([H, g_count * N], F32, tag="hT_all")
        for j in range(0, g_count * N, CH):
            w = min(CH, g_count * N - j)
            ps = ps_band.tile([H, w], F32, tag="band")
            nc.tensor.matmul(ps[:], sb["p_nemb"][:], oh4[:, j : j + w],
                             start=True, stop=True)
            nc.scalar.copy(out=hT_all[:, j : j + w], in_=ps[:])

        ident = sb["ident64"]

        # ------------------------------------------------------------------
        def egnn_layer(li, pre, n, e, hT, x4t, xt4t, rhs_slot, leading,
                       g_idx):
            """One EGNN layer. n nodes, e=n*n edges. hT (128,n) SBUF.
            x4t (n,4), xt4t (4,n). Returns (hT', x4t', xt4t', mx_psum_leftover)
            leading: list of static lhsT row segments config.
            Returns new (hT, x4t, xt4t)."""
            P = pre  # "p" or "d"
            last = li == NL - 1
            nh = n // 2 if P == "p" else n  # A'-half rows
            # --- node-level precompute -----------------------------------
            xt2 = pg.tile([3, n], F32, tag=f"{P}xt2")
            nc.vector.tensor_mul(xt2[:], xt4t[0:3, :], xt4t[0:3, :])
            # psBig: A'(half/full) at rows 0..nh, B' at rows 32..32+n (pool)
            if P == "p":
                # pool: two lhsT tiles, one per half-band
                lhs = []
                psb_copies = []
                for half in range(2):
                    psn = ps_node.tile([96, H], F32, tag="node")
                    h0 = half * 32
                    nc.tensor.matmul(psn[0:64, :], hT[:],
                                     sb[f"p{li}_we1s"][:], start=True, stop=False)
                    nc.tensor.matmul(psn[0:64, :], xt2[:],
                                     sb[f"p{li}_wd3"][:], start=False, stop=True)
                    nc.tensor.matmul(psn[64:96, :], hT[:, h0 : h0 + 32],
                                     sb[f"p{li}_we1r"][:], start=True, stop=False,
                                     tile_position=(0, 64), skip_group_check=True)
                    nc.tensor.matmul(psn[64:96, :], xt2[:, h0 : h0 + 32],
                                     sb[f"p{li}_wd3"][:], start=False, stop=True,
                                     tile_position=(0, 64), skip_group_check=True)
                    lh = pg.tile([100, H], F32, tag="lhsT")
                    nc.scalar.copy(out=lh[0:96, :], in_=psn[:])
                    nc.sync.dma_start(out=lh[96:100, :], in_=dr[f"p{li}_ewwd"][:])
                    lhs.append(lh)
            else:
                psn = ps_node.tile([48, H], F32, tag="node")
                nc.tensor.matmul(psn[0:16, :], hT[:], sb[f"d{li}_we1r"][:],
                                 start=True, stop=False)
                nc.tensor.matmul(psn[0:16, :], xt2[:], sb[f"d{li}_wd3"][:],
                                 start=False, stop=True)
                nc.tensor.matmul(psn[32:48, :], hT[:], sb[f"d{li}_we1s"][:],
                                 start=True, stop=False, tile_position=(0, 32),
                                 skip_group_check=True)
                nc.tensor.matmul(psn[32:48, :], xt2[:], sb[f"d{li}_wd3"][:],
                                 start=False, stop=True, tile_position=(0, 32),
                                 skip_group_check=True)
                lh = pg.tile([48, H], F32, tag="lhsTd")
                nc.vector.memset(lh[:], 0.0)
                nc.scalar.copy(out=lh[0:16, :], in_=psn[0:16, :])
                nc.scalar.copy(out=lh[32:48, :], in_=psn[32:48, :])
                nc.sync.dma_start(out=lh[16:17, :], in_=dr[f"d{li}_ndwd"][:])
                lhs = [lh, lh]

            # G row: X @ X.T flattened into rhs_slot
            grow_row = 99 if P == "p" else 16
            psg = ps_node.tile([n, n], F32, tag="node")
            nc.tensor.matmul(psg[:], xt4t[0:3, :], xt4t[0:3, :], start=True,
                             stop=True)
            gsb = pg.tile([n, n], F32, tag=f"{P}gsb")
            nc.scalar.copy(out=gsb[:], in_=psg[:])
            nc.sync.dma_start(
                out=rhs_slot[grow_row : grow_row + 1, :].rearrange(
                    "o (a b) -> o a b", a=n),
                in_=gsb[:].rearrange("a b -> a () b"))

            # --- edge band ------------------------------------------------
            aggT = pg.tile([H, n], F32, tag=f"{P}agg")
            nch = e // CH if e >= CH else 1
            chw = min(e, CH)
            wnat = None
            if not last:
                wnat = pg.tile([n, n], F32, tag=f"{P}wnat")
            for c in range(nch):
                k0 = c * chw
                half = 0 if P == "d" else c // (nch // 2)
                ps1 = ps_band.tile([H, chw], F32, tag="band")
                nc.tensor.matmul(ps1[:], lhs[half][:],
                                 rhs_slot[:, k0 : k0 + chw], start=True,
                                 stop=True)
                m2c = band.tile([H, chw], F32, tag="m2")
                nc.scalar.activation(out=m2c[:], in_=ps1[:], func=AF.Silu,
                                     bias=sb[f"{P}{li}_be1"][:], scale=1.0)
                ps2 = ps_band.tile([H, chw], F32, tag="band")
                nc.tensor.matmul(ps2[:], sb[f"{P}{li}_we2"][:], m2c[:],
                                 start=True, stop=True)
                mc = band.tile([H, chw], F32, tag="m")
                nc.scalar.activation(out=mc[:], in_=ps2[:], func=AF.Silu,
                                     bias=sb[f"{P}{li}_be2"][:], scale=1.0)
                # agg over senders (inner dim of k = r*n + s)
                rpc = chw // n  # receivers per chunk
                nc.vector.tensor_reduce(
                    out=aggT[:, c * rpc : (c + 1) * rpc],
                    in_=mc[:].rearrange("h (r s) -> h r s", s=n),
                    axis=AX.X, op=ALU.add)
                if not last:
                    psw = ps_w.tile([1, chw], F32, tag="w")
                    nc.tensor.matmul(psw[:], sb[f"{P}{li}_wc"][:], mc[:],
                                     start=True, stop=True)
                    wst = band.tile([1, chw], F32, tag="wst")
                    nc.vector.tensor_copy(out=wst[:], in_=psw[:])
                    rpc2 = chw // n
                    nc.sync.dma_start(
                        out=wnat[c * rpc2 : (c + 1) * rpc2, :].rearrange(
                            "a b -> a () b"),
                        in_=wst[:].rearrange("o (a b) -> o a b", a=rpc2))

            # --- coordinate update (skipped on the last layer) ------------
            if not last:
                wg = pg.tile([n, n], F32, tag=f"{P}wg")
                nc.scalar.activation(out=wg[:], in_=wnat[:], func=AF.Tanh,
                                     bias=constcol(float(_BC[P][li]))[0:n, :],
                                     scale=1.0)
                pst = ps_node.tile([n, n], F32, tag="node")
                nc.tensor.transpose(pst[:], wg[:], ident[0:n, 0:n])
                wgT = pg.tile([n, n], F32, tag=f"{P}wgT")
                nc.scalar.copy(out=wgT[:], in_=pst[:])
                psu = ps_node.tile([n, 4], F32, tag="node")
                nc.tensor.matmul(psu[:], wgT[:], x4t[:], start=True, stop=True)
                scol = pg.tile([n, 1], F32, tag=f"{P}scol")
                nc.scalar.activation(out=scol[:], in_=psu[:, 3:4],
                                     func=AF.Identity, bias=1.0,
                                     scale=-1.0 / n)
                t1 = pg.tile([n, 3], F32, tag=f"{P}t1")
                nc.vector.tensor_scalar_mul(out=t1[:], in0=x4t[:, 0:3],
                                            scalar1=scol[:])
                x4n = pg.tile([n, 4], F32, tag=f"{P}x4")
                nc.vector.scalar_tensor_tensor(out=x4n[:, 0:3], in0=psu[:, 0:3],
                                               scalar=1.0 / n, in1=t1[:],
                                               op0=ALU.mult, op1=ALU.add)
                nc.vector.memset(x4n[:, 3:4], 1.0)
                psxt = ps_node.tile([4, n], F32, tag="node")
                nc.tensor.transpose(psxt[:], x4n[:], ident[0:n, 0:n])
                xt4n = pg.tile([4, n], F32, tag=f"{P}xt4")
                nc.scalar.copy(out=xt4n[:], in_=psxt[:])
            else:
                x4n, xt4n = x4t, xt4t

            # --- node update ---------------------------------------------
            psh = ps_node.tile([H, n], F32, tag="node")
            nc.tensor.matmul(psh[:], sb[f"{P}{li}_wn1a"][:], hT[:], start=True,
                             stop=False)
            nc.tensor.matmul(psh[:], sb[f"{P}{li}_wn1b"][:], aggT[:],
                             start=False, stop=True)
            huT = pg.tile([H, n], F32, tag=f"{P}hu")
            nc.scalar.activation(out=huT[:], in_=psh[:], func=AF.Silu,
                                 bias=sb[f"{P}{li}_bn1"][:], scale=1.0)
            psh2 = ps_node.tile([H, n], F32, tag="node")
            nc.tensor.matmul(psh2[:], sb[f"{P}{li}_wn2"][:], huT[:], start=True,
                             stop=True)
            hTn = pg.tile([H, n], F32, tag=f"{P}hT")
            nc.vector.scalar_tensor_tensor(out=hTn[:], in0=psh2[:],
                                           scalar=sb[f"{P}{li}_bn2"][:],
                                           in1=hT[:], op0=ALU.add, op1=ALU.add)
            return hTn, x4n, xt4n

        # ------------------------------------------------------------------
        for g in range(g_count):
            rhs_slot = rhs_slots[g % 2]
            rhsd_slot = rhsd_slots[g % 2]

            x4_0 = pg.tile([N, 4], F32, tag="x4_0")
            nc.sync.dma_start(out=x4_0[:], in_=dr["x4"][g])
            xt4_0 = pg.tile([4, N], F32, tag="xt4_0")
            nc.sync.dma_start(out=xt4_0[:], in_=dr["xt4"][g])

            # edge-attr one-hot; computed partition-aligned then DMA'd into
            # rhs rows 96..98 (engine in/out partition ranges stay equal)
            attr_bc = wide.tile([3, E], F32, tag="attrbc")
            nc.sync.dma_start(out=attr_bc[:],
                              in_=dr["attrg"][g : g + 1, :].to_broadcast((3, E)))
            oh3 = wide.tile([3, E], F32, tag="oh3")
            nc.vector.tensor_scalar(out=oh3[:], in0=attr_bc[:],
                                    scalar1=iota3[:], scalar2=None,
                                    op0=ALU.is_equal)
            nc.sync.dma_start(out=rhs_slot[96:99, :], in_=oh3[:])

            hT = hT_all[:, g * N : (g + 1) * N]
            x4t, xt4t = x4_0, xt4_0
            for li in range(NL):
                hT, x4t, xt4t = egnn_layer(li, "p", N, E, hT, x4t, xt4t,
                                           rhs_slot, None, g)

            # ---- projection + losses ---------------------------------------
            psl = ps_node.tile([NB, N], F32, tag="node")
            nc.tensor.matmul(psl[:], sb["p_wout"][:], hT[:], start=True,
                             stop=True)
            negmax = pg.tile([NB, 1], F32, tag="negmax")
            nc.vector.tensor_reduce(out=negmax[:], in_=psl[:], axis=AX.X,
                                    op=ALU.max, negate=True)
            expT = pg.tile([NB, N], F32, tag="expT")
            nc.scalar.activation(out=expT[:], in_=psl[:], func=AF.Exp,
                                 bias=negmax[:], scale=1.0)
            srow = pg.tile([NB, 1], F32, tag="srow")
            nc.vector.tensor_reduce(out=srow[:], in_=expT[:], axis=AX.X,
                                    op=ALU.add)
            rec = pg.tile([NB, 1], F32, tag="rec")
            nc.vector.reciprocal(out=rec[:], in_=srow[:])
            projT = pg.tile([NB, N], F32, tag="projT")
            nc.vector.tensor_scalar_mul(out=projT[:], in0=expT[:], scalar1=rec[:])

            # ent: sum over everything of p*ln(p+eps) (negated on host)
            lnp = pg.tile([NB, N], F32, tag="lnp")
            nc.scalar.activation(out=lnp[:], in_=projT[:], func=AF.Ln,
                                 bias=constcol(1e-15)[0:NB, :], scale=1.0)
            pl = pg.tile([NB, N], F32, tag="pl")
            nc.vector.tensor_mul(pl[:], projT[:], lnp[:])
            nc.vector.tensor_reduce(out=scal[0:NB, 2 * g_count + g : 2 * g_count + g + 1],
                                    in_=pl[:], axis=AX.X, op=ALU.add)

            # ppt + link + diag
            pspp = ps_node.tile([N, N], F32, tag="node")
            nc.tensor.matmul(pspp[:], projT[:], projT[:], start=True, stop=True)
            dist_t = pg.tile([N, N], F32, tag="dist")
            nc.sync.dma_start(out=dist_t[:], in_=dr["dist"][g])
            dp = pg.tile([N, N], F32, tag="dp")
            nc.vector.tensor_mul(dp[:], dist_t[:], pspp[:])
            dpsq = pg.tile([N, N], F32, tag="dpsq")
            nc.scalar.activation(out=dpsq[:], in_=dp[:], func=AF.Square,
                                 accum_out=scal[:, g : g + 1])
            dg = pg.tile([N, N], F32, tag="dg")
            nc.vector.tensor_mul(dg[:], pspp[:], ident[:])
            dcol = pg.tile([N, 1], F32, tag="dcol")
            nc.vector.tensor_reduce(out=dcol[:], in_=dg[:], axis=AX.X,
                                    op=ALU.add)
            nc.scalar.activation(out=scal[:, g_count + g : g_count + g + 1],
                                 in_=dcol[:], func=AF.Relu,
                                 bias=constcol(-1.0)[0:64, :], scale=1.0)

            # ---- bead coords ------------------------------------------------
            pspj = ps_node.tile([N, NB], F32, tag="node")
            nc.tensor.transpose(pspj[:], projT[:], ident[0:NB, 0:NB])
            proj = pg.tile([N, NB], F32, tag="proj")
            nc.scalar.copy(out=proj[:], in_=pspj[:])
            psbd = ps_node.tile([NB, 4], F32, tag="node")
            nc.tensor.matmul(psbd[:], proj[:], x4_0[:], start=True, stop=True)
            bead_sb = pg.tile([NB, 4], F32, tag="beadsb")
            nc.scalar.copy(out=bead_sb[:], in_=psbd[:])
            psm = ps_node.tile([1, 4], F32, tag="node")
            nc.tensor.matmul(psm[:], sb["ones64"][0:NB, :], bead_sb[:],
                             start=True, stop=True)
            negmean = pg.tile([1, 4], F32, tag="negmean")
            nc.scalar.mul(out=negmean[:], in_=psm[:], mul=-1.0 / NB)
            nm16 = pg.tile([NB, 4], F32, tag="nm16")
            nc.gpsimd.partition_broadcast(nm16[:], negmean[:])
            bead_c = pg.tile([NB, 4], F32, tag="beadc")
            nc.vector.tensor_add(bead_c[:], bead_sb[:], nm16[:])
            nc.vector.memset(bead_c[:, 3:4], 1.0)
            psbt = ps_node.tile([4, NB], F32, tag="node")
            nc.tensor.transpose(psbt[:], bead_c[:], ident[0:NB, 0:NB])
            beadT = pg.tile([4, NB], F32, tag="beadT")
            nc.scalar.copy(out=beadT[:], in_=psbt[:])

            # ---- depool EGNN -----------------------------------------------
            hTd = sb["d_nembT"]
            x4d, xt4d = bead_c, beadT
            for li in range(NL):
                hTd, x4d, xt4d = egnn_layer(li, "d", NB, EB, hTd, x4d, xt4d,
                                            rhsd_slot, None, g)

            psi = ps_node.tile([NR, NB], F32, tag="node")
            nc.tensor.matmul(psi[:], sb["d_wout"][:], hTd[:], start=True,
                             stop=True)
            invT = pg.tile([NR, NB], F32, tag="invT")
            nc.scalar.activation(out=invT[:], in_=psi[:], func=AF.Identity,
                                 bias=sb["d_bout"][:], scale=1.0)
            psit = ps_node.tile([NB, NR], F32, tag="node")
            nc.tensor.transpose(psit[:], invT[:], ident[0:NR, 0:NR])
            inv = pg.tile([NB, NR], F32, tag="inv")
            nc.scalar.copy(out=inv[:], in_=psit[:])
            psmx = ps_node.tile([NR, 4], F32, tag="node")
            nc.tensor.matmul(psmx[:], inv[:], bead_c[:], start=True, stop=True)
            mxsb = pg.tile([NR, 3], F32, tag="mxsb")
            nc.scalar.copy(out=mxsb[:], in_=psmx[:, 0:3])
            nc.sync.dma_start(out=out_mx[g], in_=mxsb[:])

        # ---- final loss reduction ------------------------------------------
        scr = cpool.tile([64, 3], F32, tag="scr")
        nc.vector.tensor_reduce(out=scr[:],
                                in_=scal[:].rearrange("p (t g) -> p t g", g=g_count),
                                axis=AX.X, op=ALU.add)
        pssc = ps_node.tile([1, 3], F32, tag="node")
        nc.tensor.matmul(pssc[:], sb["ones64"][:], scr[:], start=True, stop=True)
        scsb = cpool.tile([1, 3], F32, tag="scsb")
        nc.scalar.copy(out=scsb[:], in_=pssc[:])
        nc.sync.dma_start(out=out_sc[:], in_=scsb[:])

    nc.compile()
    return nc


_BC = {}  # {"p": [bc_l0, bc_l1], "d": [...]} -- set before build_program


def _set_bc(inputs):
    global _BC
    _BC = {
        "p": [float(np.asarray(lp["bc"]).reshape(-1)[0])
              for lp in inputs["pool_params"]["layers"]],
        "d": [float(np.asarray(lp["bc"]).reshape(-1)[0])
              for lp in inputs["depool_params"]["layers"]],
    }


def kernel(**inputs):
    global _PROGRAM
    from concourse.bass_utils import run_bass_kernel_spmd

    _set_bc(inputs)
    if _PROGRAM is None:
        _PROGRAM = build_program(G)
    nc = _PROGRAM
    in_maps = _host_pack(inputs)
    res = run_bass_kernel_spmd(nc, in_maps, list(range(NCORES)),
                               tmpdir=os.environ.get("BASS_TMPDIR"))
    global _LAST_RESULT
    _LAST_RESULT = res
    mean_x = np.concatenate([res.results[c]["mean_x"] for c in range(NCORES)],
                            axis=0)
    sc = np.stack([res.results[c]["scalars"][0] for c in range(NCORES)])
    link_sq = sc[:, 0].astype(np.float64).sum()
    diag_sum = sc[:, 1].astype(np.float64).sum()
    ent_sum = sc[:, 2].astype(np.float64).sum()
    link_loss = np.float32(np.sqrt(link_sq))
    ent_loss = np.float32(-ent_sum / (B * NB))
    diag_loss = np.float32(diag_sum)
    return mean_x, link_loss, ent_loss, diag_loss


# revision 22
# speedup vs baseline: 1.3659x; 1.3659x over previous
"""Trainium2 Bass kernel for nn_CGAPB_84052509983238 (EGNN coarse-graining
autoencoder: pool EGNN -> softmax projection + losses -> bead coords ->
depool EGNN -> mean_x).

Sharding: data-parallel over the batch. 128 graphs / 8 cores = 16 graphs per
core; each core is fully independent (losses are partial-summed on device,
combined on host; mean_x is concatenated).

Edge layout on device: per graph the 4096 fully-connected edges are ordered
k = r*64 + s (receiver-major). The edge-MLP input M1 is assembled entirely in
PSUM by a single K=100 matmul against a mostly-static indicator rhs:
  rhs rows  0..63 : ind_s[j,k]   = [k%64 == j]           (static)
  rhs rows 64..95 : ind_r32[j,k] = [(k%2048)//64 == j]   (static)
  rhs rows 96..98 : onehot(edge_attr)                     (per graph)
  rhs row  99     : Grow[k] = (X @ X.T)[r,s]              (per graph-layer)
matched against lhsT rows [B'(64); A'half(32); EW(3); -2*wd(1)] where
A' = h@We1_r + nx2*wd, B' = h@We1_s + nx2*wd. This reproduces
M1 = h_r@We1_r + h_s@We1_s + d2*wd + emb_e@We1_e with d2 expanded as
nx2[s] + nx2[r] - 2*(x_s . x_r).
"""
import os
import sys

sys.path.insert(0, "/opt/trn_rl_repo")

import numpy as np

B, N, NB, NR, H, NL = 128, 64, 16, 32, 128, 2
NCORES = 8
G = B // NCORES  # graphs per core
E = N * N  # pool edges per graph
EB = NB * NB  # depool edges per graph
CH = 512  # band chunk (psum bank)
NCH = E // CH  # 8 chunks per pool graph-layer

_PROGRAM = None  # cached compiled program -- compile once per process
_LAST_RESULT = None  # BassKernelResults of the most recent run (for test.py)


# --------------------------------------------------------------------------
# host-side packing
# --------------------------------------------------------------------------

def _np32(a):
    return np.ascontiguousarray(np.asarray(a, dtype=np.float32))


def _col(v):
    return _np32(v).reshape(-1, 1)


def _pack_egnn(params, prefix, fold_edge0_into_be1):
    """Flatten one EGNN's params into the DRAM-tensor dict."""
    out = {}
    edge_emb = _np32(params["edge_emb"])
    for l, lp in enumerate(params["layers"]):
        We1 = _np32(lp["We1"])  # (385, H) rows: [h_r(128); h_s(128); d2(1); e(128)]
        we1r, we1s = We1[:H], We1[H : 2 * H]
        wd = We1[2 * H]  # (128,)
        we1e = We1[2 * H + 1 :]
        EW = edge_emb @ we1e  # (n_edge_types, 128)
        be1 = _np32(lp["be1"])
        if fold_edge0_into_be1:
            be1 = be1 + EW[0]
        else:
            ewwd = np.concatenate([EW, (-2.0 * wd)[None, :]], axis=0)  # (4,128)
            out[f"{prefix}{l}_ewwd"] = _np32(ewwd)
        out[f"{prefix}{l}_ndwd"] = _np32((-2.0 * wd)[None, :])  # (1,128)
        out[f"{prefix}{l}_we1r"] = we1r
        out[f"{prefix}{l}_we1s"] = we1s
        out[f"{prefix}{l}_wd3"] = _np32(np.broadcast_to(wd, (3, H)))
        out[f"{prefix}{l}_we2"] = _np32(lp["We2"])
        out[f"{prefix}{l}_wc"] = _np32(lp["Wc"])  # (128,1)
        out[f"{prefix}{l}_be1"] = _col(be1)
        out[f"{prefix}{l}_be2"] = _col(lp["be2"])
        out[f"{prefix}{l}_bn1"] = _col(lp["bn1"])
        out[f"{prefix}{l}_bn2"] = _col(lp["bn2"])
        out[f"{prefix}{l}_wn1a"] = _np32(lp["Wn1"][:H])
        out[f"{prefix}{l}_wn1b"] = _np32(lp["Wn1"][H:])
        out[f"{prefix}{l}_wn2"] = _np32(lp["Wn2"])
    return out


def _static_masks():
    k = np.arange(E)
    indp = np.zeros((96, E), np.float32)
    for j in range(64):
        indp[j] = (k % 64) == j
    for j in range(32):
        indp[64 + j] = ((k % 2048) // 64) == j
    kb = np.arange(EB)
    indd = np.zeros((48, EB), np.float32)
    for j in range(16):
        indd[j] = (kb // 16) == j
    for j in range(16):
        indd[32 + j] = (kb % 16) == j
    return indp, indd


def _host_pack(inputs, g_count=G, ncores=NCORES):
    """Build the per-core input maps."""
    x = _np32(inputs["x"]).reshape(B, N, 3)
    node_feats = np.asarray(inputs["node_feats"]).astype(np.float32).reshape(B, N)
    edge_attr = np.asarray(inputs["edge_attr"]).astype(np.float32).reshape(B, N, N)
    # reference edge k_ref = s*N + r  ->  grid[s, r]; we want attrg[r, s]
    attrg = np.ascontiguousarray(edge_attr.transpose(0, 2, 1)).reshape(B, E)
    dist = _np32(inputs["distances"])

    shared = {}
    shared.update(_pack_egnn(inputs["pool_params"], "p", fold_edge0_into_be1=False))
    shared.update(_pack_egnn(inputs["depool_params"], "d", fold_edge0_into_be1=True))
    shared["p_nemb"] = _np32(inputs["pool_params"]["node_emb"])  # (4,128)
    shared["p_wout"] = _np32(inputs["pool_params"]["Wout"])  # (128,16)
    shared["d_nembT"] = _np32(np.asarray(inputs["depool_params"]["node_emb"]).T)
    shared["d_wout"] = _np32(inputs["depool_params"]["Wout"])  # (128,32)
    shared["d_bout"] = _col(inputs["depool_params"]["bout"])  # (32,1)
    indp, indd = _static_masks()
    shared["indp"] = indp
    shared["indd"] = indd
    shared["ident64"] = np.eye(64, dtype=np.float32)
    shared["ones64"] = np.ones((64, 1), np.float32)

    x4 = np.ones((B, N, 4), np.float32)
    x4[:, :, :3] = x
    xt4 = np.ones((B, 4, N), np.float32)
    xt4[:, :3, :] = x.transpose(0, 2, 1)

    in_maps = []
    for c in range(ncores):
        sl = slice(c * g_count, (c + 1) * g_count)
        m = dict(shared)
        m["x4"] = np.ascontiguousarray(x4[sl])
        m["xt4"] = np.ascontiguousarray(xt4[sl])
        m["nf"] = np.ascontiguousarray(node_feats[sl].reshape(1, g_count * N))
        m["attrg"] = np.ascontiguousarray(attrg[sl])
        m["dist"] = np.ascontiguousarray(dist[sl])
        in_maps.append(m)
    return in_maps


# --------------------------------------------------------------------------
# device program
# --------------------------------------------------------------------------

def build_program(g_count=G):
    import concourse.bacc as bacc
    import concourse.tile as tile
    from concourse import mybir

    F32 = mybir.dt.float32
    F32R = mybir.dt.float32r
    I32 = mybir.dt.int32
    AF = mybir.ActivationFunctionType
    ALU = mybir.AluOpType
    AX = mybir.AxisListType

    nc = bacc.Bacc("TRN2", target_bir_lowering=False, debug=False,
                   num_devices=NCORES)

    def din(name, shape, dt=F32):
        return nc.dram_tensor(name, list(shape), dt, kind="ExternalInput").ap()

    def dout(name, shape, dt=F32):
        return nc.dram_tensor(name, list(shape), dt, kind="ExternalOutput").ap()

    dr = {}
    dr["x4"] = din("x4", (g_count, N, 4))
    dr["xt4"] = din("xt4", (g_count, 4, N))
    dr["nf"] = din("nf", (1, g_count * N))
    dr["attrg"] = din("attrg", (g_count, E))
    dr["dist"] = din("dist", (g_count, N, N))
    dr["indp"] = din("indp", (96, E), F32R)
    dr["indd"] = din("indd", (48, EB), F32R)
    dr["ident64"] = din("ident64", (64, 64))
    dr["ones64"] = din("ones64", (64, 1))
    dr["p_nemb"] = din("p_nemb", (4, H))
    dr["p_wout"] = din("p_wout", (H, NB))
    dr["d_nembT"] = din("d_nembT", (H, NB))
    dr["d_wout"] = din("d_wout", (H, NR))
    dr["d_bout"] = din("d_bout", (NR, 1))
    for pre in ("p", "d"):
        for l in range(NL):
            for nm, shp in [("we1r", (H, H)), ("we1s", (H, H)), ("wd3", (3, H)),
                            ("be1", (H, 1)),
                            ("be2", (H, 1)), ("bn1", (H, 1)), ("bn2", (H, 1)),
                            ("wn1a", (H, H)), ("wn1b", (H, H)), ("wn2", (H, H))]:
                dr[f"{pre}{l}_{nm}"] = din(f"{pre}{l}_{nm}", shp)
            for nm, shp in [("we2", (H, H)), ("wc", (H, 1)), ("ndwd", (1, H))]:
                dr[f"{pre}{l}_{nm}"] = din(f"{pre}{l}_{nm}", shp, F32R)
            if pre == "p":
                dr[f"p{l}_ewwd"] = din(f"p{l}_ewwd", (4, H), F32R)
    out_mx = dout("mean_x", (g_count, NR, 3))
    out_sc = dout("scalars", (1, 3))

    from contextlib import ExitStack

    with tile.TileContext(nc) as tc, ExitStack() as ctx:
        cpool = ctx.enter_context(tc.tile_pool(name="consts", bufs=1))
        pg = ctx.enter_context(tc.tile_pool(name="graph", bufs=3))
        wide = ctx.enter_context(tc.tile_pool(name="wide", bufs=2))
        band = ctx.enter_context(tc.tile_pool(name="band", bufs=4))
        ps_band = ctx.enter_context(tc.tile_pool(name="psb", bufs=2, space="PSUM"))
        ps_node = ctx.enter_context(tc.tile_pool(name="psn", bufs=2, space="PSUM"))
        ps_w = ctx.enter_context(tc.tile_pool(name="psw", bufs=2, space="PSUM"))

        # ---- load constants/params into SBUF --------------------------------
        sb = {}
        for name, ap in dr.items():
            if name in ("x4", "xt4", "nf", "attrg", "dist", "indp", "indd"):
                continue
            t = cpool.tile(list(ap.shape), ap.dtype, tag=name)
            nc.sync.dma_start(out=t[:], in_=ap[:])
            sb[name] = t

        # rhs slots for the pool band matmul (100 x 4096); rows 0..95 static
        rhs_slots = []
        for i in range(2):
            t = cpool.tile([100, E], F32R, tag=f"rhs{i}")
            nc.sync.dma_start(out=t[0:96, :], in_=dr["indp"][:])
            rhs_slots.append(t)
        # depool rhs slots (48 x 256); rows 0..15, 32..47 static, 17..31 zero
        rhsd_slots = []
        for i in range(2):
            t = cpool.tile([48, EB], F32R, tag=f"rhsd{i}")
            nc.sync.dma_start(out=t[:], in_=dr["indd"][:])
            rhsd_slots.append(t)

        # iota columns
        _ccols = {}

        def constcol(val):
            if val not in _ccols:
                t = cpool.tile([128, 1], F32, tag=f"cc{len(_ccols)}")
                nc.vector.memset(t[:], float(val))
                _ccols[val] = t
            return _ccols[val]

        iota4i = cpool.tile([4, 1], I32, tag="iota4i")
        nc.gpsimd.iota(iota4i[:], pattern=[[0, 1]], base=0, channel_multiplier=1)
        iota4 = cpool.tile([4, 1], F32, tag="iota4")
        nc.vector.tensor_copy(out=iota4[:], in_=iota4i[:])
        iota3 = iota4[0:3, :]

        # loss accumulator columns: [link(g_count) | diag | ent]
        scal = cpool.tile([64, 3 * g_count], F32, tag="scal")
        nc.vector.memset(scal[:], 0.0)

        # initial pool node features for all graphs: hT_all (128, G*64)
        nf_bc = cpool.tile([4, g_count * N], F32, tag="nfbc")
        nc.sync.dma_start(out=nf_bc[:], in_=dr["nf"].to_broadcast((4, g_count * N)))
        oh4 = cpool.tile([4, g_count * N], F32, tag="oh4")
        nc.vector.tensor_scalar(out=oh4[:], in0=nf_bc[:], scalar1=iota4[:],
                                scalar2=None, op0=ALU.is_equal)
        hT_all = cpool.tile([H, g_count * N], F32, tag="hT_all")
        for j in range(0, g_count * N, CH):
            w = min(CH, g_count * N - j)
            ps = ps_band.tile([H, w], F32, tag="band")
            nc.tensor.matmul(ps[:], sb["p_nemb"][:], oh4[:, j : j + w],
                             start=True, stop=True)
            nc.scalar.copy(out=hT_all[:, j : j + w], in_=ps[:])

        ident = sb["ident64"]

        # ------------------------------------------------------------------
        def egnn_layer(li, pre, n, e, hT, x4t, xt4t, rhs_slot, leading,
                       g_idx):
            """One EGNN layer. n nodes, e=n*n edges. hT (128,n) SBUF.
            x4t (n,4), xt4t (4,n). Returns (hT', x4t', xt4t', mx_psum_leftover)
            leading: list of static lhsT row segments config.
            Returns new (hT, x4t, xt4t)."""
            P = pre  # "p" or "d"
            last = li == NL - 1
            nh = n // 2 if P == "p" else n  # A'-half rows
            # --- node-level precompute -----------------------------------
            xt2 = pg.tile([3, n], F32, tag=f"{P}xt2")
            nc.vector.tensor_mul(xt2[:], xt4t[0:3, :], xt4t[0:3, :])
            # psBig: A'(half/full) at rows 0..nh, B' at rows 32..32+n (pool)
            if P == "p":
                # pool: two lhsT tiles, one per half-band
                lhs = []
                psb_copies = []
                for half in range(2):
                    psn = ps_node.tile([96, H], F32, tag="node")
                    h0 = half * 32
                    nc.tensor.matmul(psn[0:64, :], hT[:],
                                     sb[f"p{li}_we1s"][:], start=True, stop=False)
                    nc.tensor.matmul(psn[0:64, :], xt2[:],
                                     sb[f"p{li}_wd3"][:], start=False, stop=True)
                    nc.tensor.matmul(psn[64:96, :], hT[:, h0 : h0 + 32],
                                     sb[f"p{li}_we1r"][:], start=True, stop=False,
                                     tile_position=(0, 64), skip_group_check=True)
                    nc.tensor.matmul(psn[64:96, :], xt2[:, h0 : h0 + 32],
                                     sb[f"p{li}_wd3"][:], start=False, stop=True,
                                     tile_position=(0, 64), skip_group_check=True)
                    lh = pg.tile([100, H], F32R, tag="lhsT")
                    nc.scalar.copy(out=lh[0:96, :], in_=psn[:])
                    nc.sync.dma_start(out=lh[96:100, :], in_=dr[f"p{li}_ewwd"][:])
                    lhs.append(lh)
            else:
                psn = ps_node.tile([48, H], F32, tag="node")
                nc.tensor.matmul(psn[0:16, :], hT[:], sb[f"d{li}_we1r"][:],
                                 start=True, stop=False)
                nc.tensor.matmul(psn[0:16, :], xt2[:], sb[f"d{li}_wd3"][:],
                                 start=False, stop=True)
                nc.tensor.matmul(psn[32:48, :], hT[:], sb[f"d{li}_we1s"][:],
                                 start=True, stop=False, tile_position=(0, 32),
                                 skip_group_check=True)
                nc.tensor.matmul(psn[32:48, :], xt2[:], sb[f"d{li}_wd3"][:],
                                 start=False, stop=True, tile_position=(0, 32),
                                 skip_group_check=True)
                lh = pg.tile([48, H], F32R, tag="lhsTd")
                # rows 16..31 must be zero; indd rows 16..31 are zero
                nc.sync.dma_start(out=lh[16:32, :], in_=dr["indd"][16:32, 0:H])
                nc.scalar.copy(out=lh[0:16, :], in_=psn[0:16, :])
                nc.scalar.copy(out=lh[32:48, :], in_=psn[32:48, :])
                nc.sync.dma_start(out=lh[16:17, :], in_=dr[f"d{li}_ndwd"][:])
                lhs = [lh, lh]

            # G row: X @ X.T flattened into rhs_slot
            grow_row = 99 if P == "p" else 16
            psg = ps_node.tile([n, n], F32, tag="node")
            nc.tensor.matmul(psg[:], xt4t[0:3, :], xt4t[0:3, :], start=True,
                             stop=True)
            gsb = pg.tile([n, n], F32R, tag=f"{P}gsb")
            nc.scalar.copy(out=gsb[:], in_=psg[:])
            nc.sync.dma_start(
                out=rhs_slot[grow_row : grow_row + 1, :].rearrange(
                    "o (a b) -> o a b", a=n),
                in_=gsb[:].rearrange("a b -> a () b"))

            # --- edge band ------------------------------------------------
            # fp32r matmuls (1 cycle/col at N>=256 vs 4 for fp32); per
            # half-band the same stationary weights are grouped to cut
            # LDWEIGHTS thrash, and ACT silus run over 1024-wide tiles.
            aggT = pg.tile([H, n], F32, tag=f"{P}agg")
            QW = min(e, 1024)  # quarter width
            nhalves = 2 if P == "p" else 1
            qph = e // (QW * nhalves)  # quarters per half
            wnat = None
            if not last:
                wnat = pg.tile([n, n], F32, tag=f"{P}wnat")
            for half in range(nhalves):
                ps1s, m2s, ps2s, ms = [], [], [], []
                w0 = min(CH, QW)
                for q in range(qph):
                    k0 = half * (e // nhalves) + q * QW
                    ps1 = ps_band.tile([H, QW], F32, tag="band")
                    for j in range(QW // w0):
                        nc.tensor.matmul(
                            ps1[:, j * w0 : (j + 1) * w0],
                            lhs[half][:],
                            rhs_slot[:, k0 + j * w0 : k0 + (j + 1) * w0],
                            start=True, stop=True, skip_group_check=True)
                    ps1s.append((ps1, k0))
                for q in range(qph):
                    ps1, k0 = ps1s[q]
                    m2c = band.tile([H, QW], F32R, tag="m2")
                    nc.scalar.activation(out=m2c[:], in_=ps1[:], func=AF.Silu,
                                         bias=sb[f"{P}{li}_be1"][:], scale=1.0)
                    m2s.append(m2c)
                for q in range(qph):
                    ps2 = ps_band.tile([H, QW], F32, tag="band")
                    for j in range(QW // w0):
                        nc.tensor.matmul(
                            ps2[:, j * w0 : (j + 1) * w0],
                            sb[f"{P}{li}_we2"][:],
                            m2s[q][:, j * w0 : (j + 1) * w0],
                            start=True, stop=True, skip_group_check=True)
                    ps2s.append(ps2)
                for q in range(qph):
                    mc = band.tile([H, QW], F32R, tag="m")
                    nc.scalar.activation(out=mc[:], in_=ps2s[q][:], func=AF.Silu,
                                         bias=sb[f"{P}{li}_be2"][:], scale=1.0)
                    ms.append(mc)
                rpq = QW // n  # receivers per quarter
                for q in range(qph):
                    # agg over senders (inner dim of k = r*n + s)
                    r0 = (half * qph + q) * rpq
                    nc.vector.tensor_reduce(
                        out=aggT[:, r0 : r0 + rpq],
                        in_=ms[q][:].rearrange("h (r s) -> h r s", s=n),
                        axis=AX.X, op=ALU.add)
                if not last:
                    for q in range(qph):
                        k0 = half * (e // nhalves) + q * QW
                        for j in range(QW // w0):
                            psw = ps_w.tile([1, w0], F32, tag="w")
                            nc.tensor.matmul(
                                psw[:], sb[f"{P}{li}_wc"][:],
                                ms[q][:, j * w0 : (j + 1) * w0],
                                start=True, stop=True)
                            wst = band.tile([1, w0], F32, tag="wst")
                            nc.vector.tensor_copy(out=wst[:], in_=psw[:])
                            rr = w0 // n
                            r0 = (half * qph + q) * rpq + j * rr
                            nc.sync.dma_start(
                                out=wnat[r0 : r0 + rr, :].rearrange(
                                    "a b -> a () b"),
                                in_=wst[:].rearrange("o (a b) -> o a b", a=rr))

            # --- coordinate update (skipped on the last layer) ------------
            if not last:
                wg = pg.tile([n, n], F32, tag=f"{P}wg")
                nc.scalar.activation(out=wg[:], in_=wnat[:], func=AF.Tanh,
                                     bias=constcol(float(_BC[P][li]))[0:n, :],
                                     scale=1.0)
                pst = ps_node.tile([n, n], F32, tag="node")
                nc.tensor.transpose(pst[:], wg[:], ident[0:n, 0:n])
                wgT = pg.tile([n, n], F32, tag=f"{P}wgT")
                nc.scalar.copy(out=wgT[:], in_=pst[:])
                psu = ps_node.tile([n, 4], F32, tag="node")
                nc.tensor.matmul(psu[:], wgT[:], x4t[:], start=True, stop=True)
                scol = pg.tile([n, 1], F32, tag=f"{P}scol")
                nc.scalar.activation(out=scol[:], in_=psu[:, 3:4],
                                     func=AF.Identity, bias=1.0,
                                     scale=-1.0 / n)
                t1 = pg.tile([n, 3], F32, tag=f"{P}t1")
                nc.vector.tensor_scalar_mul(out=t1[:], in0=x4t[:, 0:3],
                                            scalar1=scol[:])
                x4n = pg.tile([n, 4], F32, tag=f"{P}x4")
                nc.vector.scalar_tensor_tensor(out=x4n[:, 0:3], in0=psu[:, 0:3],
                                               scalar=1.0 / n, in1=t1[:],
                                               op0=ALU.mult, op1=ALU.add)
                nc.vector.memset(x4n[:, 3:4], 1.0)
                psxt = ps_node.tile([4, n], F32, tag="node")
                nc.tensor.transpose(psxt[:], x4n[:], ident[0:n, 0:n])
                xt4n = pg.tile([4, n], F32, tag=f"{P}xt4")
                nc.scalar.copy(out=xt4n[:], in_=psxt[:])
            else:
                x4n, xt4n = x4t, xt4t

            # --- node update ---------------------------------------------
            psh = ps_node.tile([H, n], F32, tag="node")
            nc.tensor.matmul(psh[:], sb[f"{P}{li}_wn1a"][:], hT[:], start=True,
                             stop=False)
            nc.tensor.matmul(psh[:], sb[f"{P}{li}_wn1b"][:], aggT[:],
                             start=False, stop=True)
            huT = pg.tile([H, n], F32, tag=f"{P}hu")
            nc.scalar.activation(out=huT[:], in_=psh[:], func=AF.Silu,
                                 bias=sb[f"{P}{li}_bn1"][:], scale=1.0)
            psh2 = ps_node.tile([H, n], F32, tag="node")
            nc.tensor.matmul(psh2[:], sb[f"{P}{li}_wn2"][:], huT[:], start=True,
                             stop=True)
            hTn = pg.tile([H, n], F32, tag=f"{P}hT")
            nc.vector.scalar_tensor_tensor(out=hTn[:], in0=psh2[:],
                                           scalar=sb[f"{P}{li}_bn2"][:],
                                           in1=hT[:], op0=ALU.add, op1=ALU.add)
            return hTn, x4n, xt4n

        # ------------------------------------------------------------------
        for g in range(g_count):
            rhs_slot = rhs_slots[g % 2]
            rhsd_slot = rhsd_slots[g % 2]

            x4_0 = pg.tile([N, 4], F32, tag="x4_0")
            nc.sync.dma_start(out=x4_0[:], in_=dr["x4"][g])
            xt4_0 = pg.tile([4, N], F32, tag="xt4_0")
            nc.sync.dma_start(out=xt4_0[:], in_=dr["xt4"][g])

            # edge-attr one-hot; computed partition-aligned then DMA'd into
            # rhs rows 96..98 (engine in/out partition ranges stay equal)
            attr_bc = wide.tile([3, E], F32, tag="attrbc")
            nc.sync.dma_start(out=attr_bc[:],
                              in_=dr["attrg"][g : g + 1, :].to_broadcast((3, E)))
            oh3 = wide.tile([3, E], F32R, tag="oh3")
            nc.vector.tensor_scalar(out=oh3[:], in0=attr_bc[:],
                                    scalar1=iota3[:], scalar2=None,
                                    op0=ALU.is_equal)
            nc.sync.dma_start(out=rhs_slot[96:99, :], in_=oh3[:])

            hT = hT_all[:, g * N : (g + 1) * N]
            x4t, xt4t = x4_0, xt4_0
            for li in range(NL):
                hT, x4t, xt4t = egnn_layer(li, "p", N, E, hT, x4t, xt4t,
                                           rhs_slot, None, g)

            # ---- projection + losses ---------------------------------------
            psl = ps_node.tile([NB, N], F32, tag="node")
            nc.tensor.matmul(psl[:], sb["p_wout"][:], hT[:], start=True,
                             stop=True)
            negmax = pg.tile([NB, 1], F32, tag="negmax")
            nc.vector.tensor_reduce(out=negmax[:], in_=psl[:], axis=AX.X,
                                    op=ALU.max, negate=True)
            expT = pg.tile([NB, N], F32, tag="expT")
            nc.scalar.activation(out=expT[:], in_=psl[:], func=AF.Exp,
                                 bias=negmax[:], scale=1.0)
            srow = pg.tile([NB, 1], F32, tag="srow")
            nc.vector.tensor_reduce(out=srow[:], in_=expT[:], axis=AX.X,
                                    op=ALU.add)
            rec = pg.tile([NB, 1], F32, tag="rec")
            nc.vector.reciprocal(out=rec[:], in_=srow[:])
            projT = pg.tile([NB, N], F32, tag="projT")
            nc.vector.tensor_scalar_mul(out=projT[:], in0=expT[:], scalar1=rec[:])

            # ent: sum over everything of p*ln(p+eps) (negated on host)
            lnp = pg.tile([NB, N], F32, tag="lnp")
            nc.scalar.activation(out=lnp[:], in_=projT[:], func=AF.Ln,
                                 bias=constcol(1e-15)[0:NB, :], scale=1.0)
            pl = pg.tile([NB, N], F32, tag="pl")
            nc.vector.tensor_mul(pl[:], projT[:], lnp[:])
            nc.vector.tensor_reduce(out=scal[0:NB, 2 * g_count + g : 2 * g_count + g + 1],
                                    in_=pl[:], axis=AX.X, op=ALU.add)

            # ppt + link + diag
            pspp = ps_node.tile([N, N], F32, tag="node")
            nc.tensor.matmul(pspp[:], projT[:], projT[:], start=True, stop=True)
            dist_t = pg.tile([N, N], F32, tag="dist")
            nc.sync.dma_start(out=dist_t[:], in_=dr["dist"][g])
            dp = pg.tile([N, N], F32, tag="dp")
            nc.vector.tensor_mul(dp[:], dist_t[:], pspp[:])
            dpsq = pg.tile([N, N], F32, tag="dpsq")
            nc.scalar.activation(out=dpsq[:], in_=dp[:], func=AF.Square,
                                 accum_out=scal[:, g : g + 1])
            dg = pg.tile([N, N], F32, tag="dg")
            nc.vector.tensor_mul(dg[:], pspp[:], ident[:])
            dcol = pg.tile([N, 1], F32, tag="dcol")
            nc.vector.tensor_reduce(out=dcol[:], in_=dg[:], axis=AX.X,
                                    op=ALU.add)
            nc.scalar.activation(out=scal[:, g_count + g : g_count + g + 1],
                                 in_=dcol[:], func=AF.Relu,
                                 bias=constcol(-1.0)[0:64, :], scale=1.0)

            # ---- bead coords ------------------------------------------------
            pspj = ps_node.tile([N, NB], F32, tag="node")
            nc.tensor.transpose(pspj[:], projT[:], ident[0:NB, 0:NB])
            proj = pg.tile([N, NB], F32, tag="proj")
            nc.scalar.copy(out=proj[:], in_=pspj[:])
            psbd = ps_node.tile([NB, 4], F32, tag="node")
            nc.tensor.matmul(psbd[:], proj[:], x4_0[:], start=True, stop=True)
            bead_sb = pg.tile([NB, 4], F32, tag="beadsb")
            nc.scalar.copy(out=bead_sb[:], in_=psbd[:])
            psm = ps_node.tile([1, 4], F32, tag="node")
            nc.tensor.matmul(psm[:], sb["ones64"][0:NB, :], bead_sb[:],
                             start=True, stop=True)
            negmean = pg.tile([1, 4], F32, tag="negmean")
            nc.scalar.mul(out=negmean[:], in_=psm[:], mul=-1.0 / NB)
            nm16 = pg.tile([NB, 4], F32, tag="nm16")
            nc.gpsimd.partition_broadcast(nm16[:], negmean[:])
            bead_c = pg.tile([NB, 4], F32, tag="beadc")
            nc.vector.tensor_add(bead_c[:], bead_sb[:], nm16[:])
            nc.vector.memset(bead_c[:, 3:4], 1.0)
            psbt = ps_node.tile([4, NB], F32, tag="node")
            nc.tensor.transpose(psbt[:], bead_c[:], ident[0:NB, 0:NB])
            beadT = pg.tile([4, NB], F32, tag="beadT")
            nc.scalar.copy(out=beadT[:], in_=psbt[:])

            # ---- depool EGNN -----------------------------------------------
            hTd = sb["d_nembT"]
            x4d, xt4d = bead_c, beadT
            for li in range(NL):
                hTd, x4d, xt4d = egnn_layer(li, "d", NB, EB, hTd, x4d, xt4d,
                                            rhsd_slot, None, g)

            psi = ps_node.tile([NR, NB], F32, tag="node")
            nc.tensor.matmul(psi[:], sb["d_wout"][:], hTd[:], start=True,
                             stop=True)
            invT = pg.tile([NR, NB], F32, tag="invT")
            nc.scalar.activation(out=invT[:], in_=psi[:], func=AF.Identity,
                                 bias=sb["d_bout"][:], scale=1.0)
            psit = ps_node.tile([NB, NR], F32, tag="node")
            nc.tensor.transpose(psit[:], invT[:], ident[0:NR, 0:NR])
            inv = pg.tile([NB, NR], F32, tag="inv")
            nc.scalar.copy(out=inv[:], in_=psit[:])
            psmx = ps_node.tile([NR, 4], F32, tag="node")
            nc.tensor.matmul(psmx[:], inv[:], bead_c[:], start=True, stop=True)
            mxsb = pg.tile([NR, 3], F32, tag="mxsb")
            nc.scalar.copy(out=mxsb[:], in_=psmx[:, 0:3])
            nc.sync.dma_start(out=out_mx[g], in_=mxsb[:])

        # ---- final loss reduction ------------------------------------------
        scr = cpool.tile([64, 3], F32, tag="scr")
        nc.vector.tensor_reduce(out=scr[:],
                                in_=scal[:].rearrange("p (t g) -> p t g", g=g_count),
                                axis=AX.X, op=ALU.add)
        pssc = ps_node.tile([1, 3], F32, tag="node")
        nc.tensor.matmul(pssc[:], sb["ones64"][:], scr[:], start=True, stop=True)
        scsb = cpool.tile([1, 3], F32, tag="scsb")
        nc.scalar.copy(out=scsb[:], in_=pssc[:])
        nc.sync.dma_start(out=out_sc[:], in_=scsb[:])

    nc.compile()
    return nc


_BC = {}  # {"p": [bc_l0, bc_l1], "d": [...]} -- set before build_program


def _set_bc(inputs):
    global _BC
    _BC = {
        "p": [float(np.asarray(lp["bc"]).reshape(-1)[0])
              for lp in inputs["pool_params"]["layers"]],
        "d": [float(np.asarray(lp["bc"]).reshape(-1)[0])
              for lp in inputs["depool_params"]["layers"]],
    }


def kernel(**inputs):
    global _PROGRAM
    from concourse.bass_utils import run_bass_kernel_spmd

    _set_bc(inputs)
    if _PROGRAM is None:
        _PROGRAM = build_program(G)
    nc = _PROGRAM
    in_maps = _host_pack(inputs)
    res = run_bass_kernel_spmd(nc, in_maps, list(range(NCORES)),
                               tmpdir=os.environ.get("BASS_TMPDIR"))
    global _LAST_RESULT
    _LAST_RESULT = res
    mean_x = np.concatenate([res.results[c]["mean_x"] for c in range(NCORES)],
                            axis=0)
    sc = np.stack([res.results[c]["scalars"][0] for c in range(NCORES)])
    link_sq = sc[:, 0].astype(np.float64).sum()
    diag_sum = sc[:, 1].astype(np.float64).sum()
    ent_sum = sc[:, 2].astype(np.float64).sum()
    link_loss = np.float32(np.sqrt(link_sq))
    ent_loss = np.float32(-ent_sum / (B * NB))
    diag_loss = np.float32(diag_sum)
    return mean_x, link_loss, ent_loss, diag_loss
